# revision 1
# baseline (speedup 1.0000x reference)
"""Trainium2 Bass kernel for nn_Discriminator (dense MLP + pairwise L1 diversity).

SPMD over 8 cores. Dense layers are data-parallel over the N=1024 rows
(128 rows per core, activations kept feature-major for the PE). The
diversity term

    div[j,k] = sum_i exp( - sum_d |M[i,k,d] - M[j,k,d]| ),  M = h @ Wd + bd

uses the exact identity |B - s| = 2*relu(B - s) - B + s, so per (k,d):
  - DVE tensor_scalar(subtract, max): A = relu(B - s), bf16 4x mode
  - PE: identity matmuls accumulate A over d into PSUM, plus one K=1
    ones-row matmul adding -Sb/2[i] (Sb[i] = sum_d M[i,kd]); 2 of 10
    d-streams are pre-added pairwise on DVE to balance the engines
  - ACT: one activation(Exp, scale=-2, bias=-Ss[j], accum_out=...) fuses
    the exponential and the row-sum over i into the concat tile's column.

Work split stays core-uniform (one program for all cores): core c handles
kernel c for all eight 128-row blocks (its B tiles amortize 8x) plus
kernels 8 and 9 for its own block. Core identity enters only through the
collectives: an AllToAll of M^T rows 0..79 with 10-row shards hands each
core its own kernel's rows; an AllGather supplies rows 80..99; a second
AllToAll returns the div columns to their row owners. B tiles are DMA
row-broadcasts from DRAM (kernels 8/9 via gpsimd.partition_broadcast on
the otherwise idle Pool engine). M travels in bf16; scalars/PSUM/LN stay
fp32 (rel err ~3e-3 from the single consistent M quantization).
"""

import os
import sys

import numpy as np

sys.path.insert(0, "/opt/trn_rl_repo")

import concourse.bass as bass
import concourse.bacc as bacc
import concourse.tile as tile
from concourse import mybir
from concourse.bass_utils import run_bass_kernel_spmd

try:
    import ml_dtypes

    BF16_NP = ml_dtypes.bfloat16
except ImportError:  # pragma: no cover
    BF16_NP = None

F32 = mybir.dt.float32
BF16 = mybir.dt.bfloat16

N = 1024
NF = 512
HID = 256
NK = 10
KD = 10
MB = NK * KD  # 100
CAT = HID + NK  # 266
EPS = 1e-3
ALPHA = 0.3
NCORES = 8
P = N // NCORES  # 128 rows per core

AF = mybir.ActivationFunctionType
ALU = mybir.AluOpType


def _chunks(total, size):
    out = []
    o = 0
    while o < total:
        out.append((o, min(size, total - o)))
        o += size
    return out


def build_program(stage="full"):
    nc = bacc.Bacc(
        "TRN2",
        target_bir_lowering=False,
        debug=False,
        num_devices=NCORES,
    )

    # ---- per-core external inputs ----
    xT = nc.dram_tensor("xT", [NF, P], F32, kind="ExternalInput")
    W0 = nc.dram_tensor("W0", [NF, HID], F32, kind="ExternalInput")
    b0c = nc.dram_tensor("b0c", [HID, 1], F32, kind="ExternalInput")
    Wd0 = nc.dram_tensor("Wd0", [HID, MB], F32, kind="ExternalInput")
    bd0c = nc.dram_tensor("bd0c", [MB, 1], F32, kind="ExternalInput")
    beta0b = nc.dram_tensor("beta0b", [P, CAT], F32, kind="ExternalInput")
    W1 = nc.dram_tensor("W1", [CAT, HID], F32, kind="ExternalInput")
    b1c = nc.dram_tensor("b1c", [HID, 1], F32, kind="ExternalInput")
    Wd1 = nc.dram_tensor("Wd1", [HID, MB], F32, kind="ExternalInput")
    bd1c = nc.dram_tensor("bd1c", [MB, 1], F32, kind="ExternalInput")
    beta1b = nc.dram_tensor("beta1b", [P, CAT], F32, kind="ExternalInput")
    Wfb = nc.dram_tensor("Wfb", [P, CAT], F32, kind="ExternalInput")
    bfc = nc.dram_tensor("bfc", [P, 1], F32, kind="ExternalInput")

    # per-core one-hot [100, 10]: column m selects M^T row 10*core + m
    Ssel = nc.dram_tensor("Ssel", [MB, NK], BF16, kind="ExternalInput")

    y_out = nc.dram_tensor("y", [P, 1], F32, kind="ExternalOutput")

    # ---- NEFF-embedded constants ----
    ident_f32 = nc.inline_tensor(np.eye(128, dtype=np.float32), name="ident_f32")
    ident_bf16 = nc.inline_tensor(
        np.eye(128).astype(BF16_NP), name="ident_bf16"
    )
    ones1_f32 = nc.inline_tensor(
        np.ones((1, 128), dtype=np.float32), name="ones1_f32"
    )
    # column sums with -0.5 scaling for the Sb rows
    nh10_c = nc.inline_tensor(
        np.full((KD, 1), -0.5).astype(BF16_NP), name="nh10"
    )
    _nh2 = np.zeros((2 * KD, 2))
    _nh2[:KD, 0] = -0.5
    _nh2[KD:, 1] = -0.5
    nh20x2_c = nc.inline_tensor(_nh2.astype(BF16_NP), name="nh20x2")

    with tile.TileContext(nc, num_cores=NCORES) as tc:
        dram = tc.alloc_tile_pool(name="dram", bufs=1, space="DRAM")
        m_loc = [dram.tile([MB, P], BF16, name=f"m_loc{b}") for b in range(2)]
        m_gath = [
            dram.tile(
                [NCORES, MB, P], BF16,
                addr_space=("Local" if stage == "nocc" else "Shared"),
                name=f"m_gath{b}",
            )
            for b in range(2)
        ]
        # rows 80..99 of M^T (kernels 8, 9) and the selected kernel rows
        mt89_dram = [dram.tile([2 * KD, N], BF16, name=f"mt89_d{b}") for b in range(2)]
        mtA_dram = [dram.tile([KD, N], BF16, name=f"mtA_d{b}") for b in range(2)]
        # AllToAll of M^T rows 0..79: shard c = rows of kernel c, so every
        # core receives its own kernel's rows from all peers (1/10th the
        # AllGather payload, and off the mt_sb assembly path)
        mtam_recv = [
            dram.tile([NCORES, KD, P], BF16, name=f"mtam_r{b}") for b in range(2)
        ]
        a2a_send = [dram.tile([NCORES, P], F32, name=f"a2a_s{b}") for b in range(2)]
        a2a_recv = [
            dram.tile([NCORES, P], F32, name=f"a2a_r{b}") for b in range(2)
        ]
        consts = tc.alloc_tile_pool(name="consts", bufs=1)
        acts = tc.alloc_tile_pool(name="acts", bufs=1)
        mtiles = tc.alloc_tile_pool(name="mtiles", bufs=2)
        bpool = tc.alloc_tile_pool(name="bpool", bufs=2)
        apool = tc.alloc_tile_pool(name="apool", bufs=6)
        epool = tc.alloc_tile_pool(name="epool", bufs=2)
        rows = tc.alloc_tile_pool(name="rows", bufs=1)
        small = tc.alloc_tile_pool(name="small", bufs=4)
        ps_small = tc.alloc_tile_pool(name="ps_small", bufs=2, space="PSUM")
        ps_l1 = tc.alloc_tile_pool(name="ps_l1", bufs=3, space="PSUM")

        # ---------- load constants ----------
        # startup-critical consts via HWDGE (sync); only the late-needed
        # block-1/LN/head weights ride the Pool queue, few enough that the
        # M-chain DMAs queued behind them are not delayed
        def load(dram, shape, dtype=F32, name=None, late=False):
            t = consts.tile(shape, dtype, name=name)
            (nc.gpsimd if late else nc.sync).dma_start(out=t, in_=dram)
            return t

        xT_sb = [
            load(xT[o : o + sz, :], [sz, P], name=f"xT{i}")
            for i, (o, sz) in enumerate(_chunks(NF, 128))
        ]
        w0_sb = [
            load(W0[o : o + sz, :], [sz, HID], name=f"w0_{i}")
            for i, (o, sz) in enumerate(_chunks(NF, 128))
        ]
        idf = load(ident_f32[:, :], [128, 128], name="idf")
        idb = load(ident_bf16[:, :], [128, 128], BF16, name="idb")
        ones1 = load(ones1_f32[:, :], [1, 128], name="ones1")
        nh10 = load(nh10_c[:, :], [KD, 1], BF16, name="nh10")
        nh20x2 = load(nh20x2_c[:, :], [2 * KD, 2], BF16, name="nh20x2")
        w1_sb = [
            load(W1[o : o + sz, :], [sz, HID], name=f"w1_{i}", late=True)
            for i, (o, sz) in enumerate(_chunks(CAT, 128))
        ]
        wd0_sb = [
            load(Wd0[o : o + sz, :], [sz, MB], name=f"wd0_{i}")
            for i, (o, sz) in enumerate(_chunks(HID, 128))
        ]
        wd1_sb = [
            load(Wd1[o : o + sz, :], [sz, MB], name=f"wd1_{i}", late=True)
            for i, (o, sz) in enumerate(_chunks(HID, 128))
        ]
        b0_sb = [
            load(b0c[o : o + sz, :], [sz, 1], name=f"b0_{i}")
            for i, (o, sz) in enumerate(_chunks(HID, 128))
        ]
        b1_sb = [
            load(b1c[o : o + sz, :], [sz, 1], name=f"b1_{i}", late=True)
            for i, (o, sz) in enumerate(_chunks(HID, 128))
        ]
        bd0_sb = load(bd0c[:, :], [MB, 1], name="bd0")
        bd1_sb = load(bd1c[:, :], [MB, 1], name="bd1", late=True)
        beta_sb = [
            load(beta0b[:, :], [P, CAT], name="beta0", late=True),
            load(beta1b[:, :], [P, CAT], name="beta1", late=True),
        ]
        wf_sb = load(Wfb[:, :], [P, CAT], name="wf", late=True)
        bf_sb = load(bfc[:, :], [P, 1], name="bf", late=True)

        eps_sb = consts.tile([P, 1], F32, name="eps")
        nc.vector.memset(eps_sb, EPS)

        # ---------- one block ----------
        def block(b, prevT, w_sb, b_sb, wd_sb, bd_sb, do_div=True, upto=None):
            """prevT: list of (tile, psize) feature-major chunks of the input.

            Returns cat tile [P, CAT] = LeakyReLU(LN(concat(h, div))).
            """
            # h^T = W^T @ prev + b   (feature-major, HID x P as 2 chunks)
            hT = []
            for mi, (mo, msz) in enumerate(_chunks(HID, 128)):
                ps = ps_small.tile([128, P], F32, tag="ps_small")
                for ki, (wt, (pt, psz)) in enumerate(zip(w_sb, prevT)):
                    nc.tensor.matmul(
                        ps[:msz, :],
                        wt[:, mo : mo + msz],
                        pt,
                        start=(ki == 0),
                        stop=(ki == len(w_sb) - 1),
                    )
                ht = acts.tile([msz, P], F32, name=f"hT{b}_{mi}")
                nc.vector.tensor_scalar(
                    out=ht, in0=ps[:msz, :], scalar1=b_sb[mi], scalar2=None,
                    op0=ALU.add,
                )
                hT.append((ht, msz))
            if upto == "h":
                return hT[0][0]

            # M^T = Wd^T @ h + bd   [100, 128]
            ps_m = ps_small.tile([MB, P], F32, tag="ps_small")
            for ki, ((ht, _), wdt) in enumerate(zip(hT, wd_sb)):
                nc.tensor.matmul(
                    ps_m,
                    wdt,
                    ht,
                    start=(ki == 0),
                    stop=(ki == len(wd_sb) - 1),
                )
            mT = mtiles.tile([MB, P], F32, tag="mT")
            nc.vector.tensor_scalar(
                out=mT, in0=ps_m, scalar1=bd_sb, scalar2=None, op0=ALU.add
            )

            # own M rows (row-major, fp32) for per-partition scalars
            ps_t = ps_small.tile([128, MB], F32, tag="ps_small")
            nc.tensor.transpose(ps_t[:, :], mT, idf[:MB, :MB])
            m_row = mtiles.tile([P, MB], F32, tag="m_row")
            nc.vector.tensor_copy(m_row, ps_t[:, :MB])
            if upto == "m":
                return m_row

            # concat tile; div columns are filled by the diversity loop
            cat = acts.tile([P, CAT], F32, name=f"cat{b}")
            if not do_div:
                nc.vector.memset(cat[:, HID:CAT], 1.0)

            # ---- gather M^T and build per-core slices ----
            # unit u=0..7: (kernel = sel-core, J-block = u)
            # unit u=8, 9: (kernel 8/9, J-block = own rows)
            if do_div:
                # SWDGE casts f32 -> bf16 during the transfer; no DVE copy
                nc.gpsimd.dma_start(out=m_loc[b][:, :], in_=mT)
                if stage == "nocc":
                    nc.gpsimd.dma_start(
                        out=mtam_recv[b][:, :, :], in_=m_loc[b][0:80, :]
                    )
                    for c in range(NCORES):
                        nc.sync.dma_start(
                            out=m_gath[b][c, :, :], in_=m_loc[b][:, :]
                        )
                else:
                    nc.gpsimd.collective_compute(
                        "AllToAll",
                        ALU.bypass,
                        replica_groups=[list(range(NCORES))],
                        ins=[m_loc[b][0:80, :]],
                        outs=[mtam_recv[b][:, :, :]],
                    )
                    nc.gpsimd.collective_compute(
                        "AllGather",
                        ALU.bypass,
                        replica_groups=[list(range(NCORES))],
                        ins=[m_loc[b][:, :]],
                        outs=[m_gath[b][:, :, :]],
                    )
                # one DMA assembles [100, 1024] from the gathered blocks
                mt_sb = mtiles.tile([MB, N], BF16, tag="mt_sb")
                gsrc = m_gath[b][:, :, :]
                gath_ap = bass.AP(
                    tensor=gsrc.tensor,
                    offset=gsrc.offset,
                    ap=[[P, MB], [MB * P, NCORES], [1, P]],
                )
                nc.gpsimd.dma_start(out=mt_sb, in_=gath_ap)
                # kernels 8,9 rows -> DRAM (for broadcast) and base-0 SBUF
                nc.gpsimd.dma_start(out=mt89_dram[b][:, :], in_=mt_sb[80:100, :])
                mt89_sb = mtiles.tile([2 * KD, N], BF16, tag="mt89_sb")
                nc.gpsimd.dma_start(out=mt89_sb, in_=mt89_dram[b][:, :])
                # same 20 rows flattened onto partition 0 (partition_broadcast
                # sources must start at partition 0)
                mt89_row = rows.tile([1, 2 * KD * N], BF16, tag="mt89_row")
                nc.gpsimd.dma_start(
                    out=mt89_row,
                    in_=bass.AP(
                        tensor=mt89_dram[b][:, :].tensor,
                        offset=mt89_dram[b][:, :].offset,
                        ap=[[0, 1], [1, 2 * KD * N]],
                    ),
                )
                # own kernel's rows, assembled from the AllToAll result.
                # Two independent hops off the same source: DRAM->DRAM for the
                # broadcast source, DRAM->SBUF for negSb/scalars — parallel,
                # so the broadcast doesn't wait on the SBUF round-trip.
                rsrc = mtam_recv[b][:, :, :]
                asm_ap = bass.AP(
                    tensor=rsrc.tensor,
                    offset=rsrc.offset,
                    ap=[[P, KD], [KD * P, NCORES], [1, P]],
                )
                nc.gpsimd.dma_start(out=mtA_dram[b][:, :], in_=asm_ap)
                mtA_sb = mtiles.tile([KD, N], BF16, tag="mtA_sb")
                nc.gpsimd.dma_start(out=mtA_sb, in_=asm_ap)

                # -Sb/2 rows ([1, N] fp32 at partition 0) for the 3 kernels
                def sbrow(lhsT, rhs_sb, nm):
                    row = rows.tile([1, N], F32, tag=nm)
                    for ho, hsz in _chunks(N, 512):
                        ps_r = ps_small.tile([1, 512], F32, tag="ps_small")
                        nc.tensor.matmul(
                            ps_r[:, :hsz], lhsT, rhs_sb[:, ho : ho + hsz],
                            start=True, stop=True,
                        )
                        nc.scalar.activation(
                            row[:, ho : ho + hsz], ps_r[:, :hsz], AF.Copy,
                            bias=0.0, scale=1.0,
                        )
                    return row

                negsbA = sbrow(nh10, mtA_sb, "negsbA")
                negsb8 = sbrow(nh20x2[:, 0:1], mt89_sb, "negsb8")
                negsb9 = sbrow(nh20x2[:, 1:2], mt89_sb, "negsb9")

                # broadcast mega-tiles [128, 10*N]: same row set on every
                # partition (DMA reads the DRAM rows 128 times)
                def bmega(dram_ap, nm):
                    bt = bpool.tile([P, KD * N], BF16, tag="bt")
                    bcast = bass.AP(
                        tensor=dram_ap.tensor,
                        offset=dram_ap.offset,
                        ap=[[0, P], [1, KD * N]],
                    )
                    nc.gpsimd.dma_start(out=bt, in_=bcast)
                    return bt

                # split the broadcast so unit 0 can start after the first
                # two d-slices land instead of the full 2.5 MB
                btA0 = bpool.tile([P, 2 * N], BF16, tag="btA0")
                src0 = mtA_dram[b][0:1, :]
                nc.gpsimd.dma_start(
                    out=btA0,
                    in_=bass.AP(
                        tensor=src0.tensor, offset=src0.offset,
                        ap=[[0, P], [1, 2 * N]],
                    ),
                )
                btA1 = bpool.tile([P, (KD - 2) * N], BF16, tag="btA1")
                src1 = mtA_dram[b][2:3, :]
                nc.gpsimd.dma_start(
                    out=btA1,
                    in_=bass.AP(
                        tensor=src1.tensor, offset=src1.offset,
                        ap=[[0, P], [1, (KD - 2) * N]],
                    ),
                )

                # kernels 8/9: broadcast on the (otherwise idle) Pool engine
                # straight from SBUF; needed only at the end of the unit loop
                def bmega_pool(row0):
                    bt = bpool.tile([P, KD * N], BF16, tag="bt")
                    for d in range(KD):
                        nc.gpsimd.partition_broadcast(
                            bt[:, d * N : (d + 1) * N],
                            mt89_row[0:1, (row0 + d) * N : (row0 + d + 1) * N],
                        )
                    return bt

                bt8 = bmega_pool(0)
                bt9 = bmega_pool(KD)

                divsend = acts.tile([P, NCORES], F32, name=f"divsend{b}")

                for u in range(NK):
                    if u < NCORES:
                        negsb = negsbA

                        def bt_slice(d):
                            if d < 2:
                                return btA0[:, d * N : (d + 1) * N]
                            return btA1[:, (d - 2) * N : (d - 1) * N]
                        # scalars: M[J-block u rows, own-kernel cols] =
                        # transpose of the mtA slice for block u
                        ps_sc = ps_small.tile([128, KD], BF16, tag="ps_small")
                        nc.tensor.transpose(
                            ps_sc[:, :KD],
                            mtA_sb[:, u * P : (u + 1) * P],
                            idb[:KD, :KD],
                        )
                        scal = small.tile([P, KD], F32, tag="scal")
                        nc.vector.tensor_copy(scal, ps_sc[:, :KD])
                        accum_dst = divsend[:, u : u + 1]
                    else:
                        bt = bt8 if u == 8 else bt9
                        negsb = negsb8 if u == 8 else negsb9

                        def bt_slice(d, _bt=bt):
                            return _bt[:, d * N : (d + 1) * N]
                        scal = small.tile([P, KD], F32, tag="scal")
                        nc.vector.tensor_copy(
                            scal, m_row[:, (u - 8 + 8) * KD : (u - 7 + 8) * KD]
                        )
                        accum_dst = cat[:, HID + u : HID + u + 1]
                    nss = small.tile([P, 1], F32, tag="nss")
                    nc.vector.tensor_reduce(
                        out=nss, in_=scal, axis=mybir.AxisListType.X,
                        op=ALU.add, negate=True,
                    )
                    psl = ps_l1.tile([P, N], F32, tag="psl")

                    def relu_d(d):
                        at = apool.tile([P, N], BF16, tag="at")
                        nc.vector.tensor_scalar(
                            out=at,
                            in0=bt_slice(d),
                            scalar1=scal[:, d : d + 1],
                            scalar2=0.0,
                            op0=ALU.subtract,
                            op1=ALU.max,
                        )
                        return at

                    def stream(at, first):
                        for ho, hsz in _chunks(N, 512):
                            nc.tensor.matmul(
                                psl[:, ho : ho + hsz],
                                idb,
                                at[:, ho : ho + hsz],
                                start=first,
                                stop=False,
                            )

                    # d = 0..5 stream straight into PSUM; d = 6..9 are
                    # pre-added pairwise on DVE to offload the PE
                    for d in range(6):
                        stream(relu_d(d), d == 0)
                    for lo in (6, 8):
                        a0, a1 = relu_d(lo), relu_d(lo + 1)
                        comb = apool.tile([P, N], BF16, tag="comb")
                        nc.vector.tensor_add(comb, a0, a1)
                        stream(comb, False)
                    for ho, hsz in _chunks(N, 512):
                        nc.tensor.matmul(
                            psl[:, ho : ho + hsz],
                            ones1,
                            negsb[:, ho : ho + hsz],
                            start=False,
                            stop=True,
                        )
                    escr = epool.tile([P, N], BF16, tag="escr")
                    nc.scalar.activation(
                        escr, psl, AF.Exp, bias=nss, scale=-2.0,
                        accum_out=accum_dst,
                    )

                # exchange div columns: shard u of our send buffer holds the
                # result for core u; AllToAll routes sender k's shard c to
                # slot k on core c  ->  recv[k] = div[own rows, kernel k]
                ps_ds = ps_small.tile([128, P], F32, tag="ps_small")
                nc.tensor.transpose(ps_ds[:NCORES, :], divsend, idf)
                dsend_sb = small.tile([NCORES, P], F32, tag="dsend")
                nc.vector.tensor_copy(dsend_sb, ps_ds[:NCORES, :])
                nc.gpsimd.dma_start(out=a2a_send[b][:, :], in_=dsend_sb)
                if stage == "nocc":
                    nc.gpsimd.dma_start(
                        out=a2a_recv[b][:, :], in_=a2a_send[b][:, :]
                    )
                else:
                    nc.gpsimd.collective_compute(
                        "AllToAll",
                        ALU.bypass,
                        replica_groups=[list(range(NCORES))],
                        ins=[a2a_send[b][:, :]],
                        outs=[a2a_recv[b][:, :]],
                    )
                drecv_sb = small.tile([NCORES, P], F32, tag="drecv")
                nc.gpsimd.dma_start(out=drecv_sb, in_=a2a_recv[b][:, :])
                ps_dr = ps_small.tile([128, NCORES], F32, tag="ps_small")
                nc.tensor.transpose(
                    ps_dr[:, :NCORES], drecv_sb, idf[:NCORES, :NCORES]
                )
                nc.vector.tensor_copy(
                    cat[:, HID : HID + NCORES], ps_dr[:, :NCORES]
                )

            # h rows into cat[:, :256] via PE transposes of hT
            for mi, (ht, msz) in enumerate(hT):
                ps_t2 = ps_small.tile([128, P], F32, tag="ps_small")
                nc.tensor.transpose(ps_t2[:, :msz], ht, idf[:msz, :msz])
                nc.vector.tensor_copy(
                    cat[:, mi * 128 : mi * 128 + msz], ps_t2[:, :msz]
                )

            if upto == "cat":
                return cat
            # LayerNorm (center+scale, beta only) + LeakyReLU
            stats = small.tile([P, 6], F32, tag="stats")
            nc.vector.bn_stats(out=stats, in_=cat)
            mv = small.tile([P, 2], F32, tag="mv")
            nc.vector.bn_aggr(out=mv, in_=stats)
            rstd = small.tile([P, 1], F32, tag="rstd")
            nc.scalar.activation(
                rstd, mv[:, 1:2], AF.Sqrt, bias=eps_sb, scale=1.0
            )
            nc.vector.reciprocal(out=rstd, in_=rstd)
            if upto == "stats":
                return mv
            catn = acts.tile([P, CAT], F32, name=f"catn{b}")
            nc.vector.tensor_scalar(
                out=catn,
                in0=cat,
                scalar1=mv[:, 0:1],
                scalar2=rstd,
                op0=ALU.subtract,
                op1=ALU.mult,
            )
            nc.vector.tensor_add(catn, catn, beta_sb[b])
            if upto == "ln":
                return catn
            # leaky relu: max(x, 0.3x)
            scr = acts.tile([P, CAT], F32, name=f"lrelu{b}")
            nc.scalar.activation(scr, catn, AF.Copy, bias=0.0, scale=ALPHA)
            hout = acts.tile([P, CAT], F32, name=f"hout{b}")
            nc.vector.tensor_tensor(
                out=hout, in0=catn, in1=scr, op=ALU.max
            )
            if upto == "lrelu":
                return hout
            return hout

        # ---------- block 0 ----------
        prev0 = [(t, 128) for t in xT_sb]
        upto = stage if stage in ("h", "m", "cat", "stats", "ln", "lrelu") else None
        h1 = block(0, prev0, w0_sb, b0_sb, wd0_sb, bd0_sb,
                   do_div=(stage in ("full", "b0", "nocc")), upto=upto)
        if upto is not None:
            ytmp = small.tile([P, 1], F32, tag="ysb")
            nc.vector.tensor_copy(ytmp, h1[:, 0:1])
            nc.sync.dma_start(out=y_out[:, :], in_=ytmp)
            h1 = None

        if upto is not None:
            pass
        elif stage in ("full", "nocc"):
            # transpose h1 -> feature-major chunks for block 1
            h1T = []
            for ci, (co, csz) in enumerate(_chunks(CAT, 128)):
                ps_t = ps_small.tile([128, P], F32, tag="ps_small")
                nc.tensor.transpose(ps_t[:csz, :], h1[:, co : co + csz], idf)
                ht = acts.tile([csz, P], F32, name=f"h1T_{ci}")
                nc.vector.tensor_copy(ht, ps_t[:csz, :])
                h1T.append((ht, csz))

            # ---------- block 1 ----------
            h2 = block(1, h1T, w1_sb, b1_sb, wd1_sb, bd1_sb)
        else:
            h2 = h1

        # ---------- critic head: y = h2 @ Wf + bf ----------
        if upto is None:
            hw = acts.tile([P, CAT], F32, name="hw")
            yacc = small.tile([P, 1], F32, tag="yacc")
            nc.vector.tensor_mul(hw, h2, wf_sb)
            nc.vector.tensor_reduce(
                out=yacc, in_=hw, axis=mybir.AxisListType.X, op=ALU.add
            )
            ysb = small.tile([P, 1], F32, tag="ysb")
            nc.scalar.activation(ysb, yacc, AF.Identity, bias=bf_sb, scale=1.0)
            nc.sync.dma_start(out=y_out[:, :], in_=ysb)

        ps_l1.release()
        ps_small.release()
        small.release()
        rows.release()
        epool.release()
        apool.release()
        bpool.release()
        mtiles.release()
        acts.release()
        consts.release()
        dram.release()

    nc.compile()
    return nc


_NC_CACHE = {}


def _get_nc():
    stage = os.environ.get("KERNEL_STAGE", "full")
    if stage not in _NC_CACHE:
        _NC_CACHE[stage] = build_program(stage)
    return _NC_CACHE[stage]


def _make_in_maps(inputs):
    f = lambda a: np.ascontiguousarray(np.asarray(a, dtype=np.float32))
    x = f(inputs["x"])
    shared = {
        "W0": f(inputs["W0"]),
        "b0c": f(inputs["b0"]).reshape(HID, 1),
        "Wd0": f(inputs["Wd0"]),
        "bd0c": f(inputs["bd0"]).reshape(MB, 1),
        "beta0b": np.ascontiguousarray(
            np.broadcast_to(f(inputs["beta0"]), (P, CAT))
        ),
        "W1": f(inputs["W1"]),
        "b1c": f(inputs["b1"]).reshape(HID, 1),
        "Wd1": f(inputs["Wd1"]),
        "bd1c": f(inputs["bd1"]).reshape(MB, 1),
        "beta1b": np.ascontiguousarray(
            np.broadcast_to(f(inputs["beta1"]), (P, CAT))
        ),
        "Wfb": np.ascontiguousarray(
            np.broadcast_to(f(inputs["Wf"]).reshape(1, CAT), (P, CAT))
        ),
        "bfc": np.full((P, 1), float(np.asarray(inputs["bf"]).reshape(-1)[0]),
                       dtype=np.float32),
    }
    if BF16_NP is None:
        raise RuntimeError("ml_dtypes required for bf16 inputs")
    in_maps = []
    for c in range(NCORES):
        m = dict(shared)
        m["xT"] = np.ascontiguousarray(x[c * P : (c + 1) * P, :].T)
        sel = np.zeros((MB, NK), dtype=np.float32)
        for j in range(NK):
            sel[(10 * c + j) % MB, j] = 1.0
        m["Ssel"] = sel.astype(BF16_NP)
        in_maps.append(m)
    return in_maps


def run(inputs, **kw):
    nc = _get_nc()
    in_maps = _make_in_maps(inputs)
    res = run_bass_kernel_spmd(nc, in_maps, list(range(NCORES)), **kw)
    y = np.concatenate([res.results[c]["y"] for c in range(NCORES)], axis=0)
    return y.astype(np.float32), res


def kernel(**inputs) -> np.ndarray:
    y, _ = run(inputs)
    return y



# revision 3
# speedup vs baseline: 1.0111x; 1.0111x over previous
"""Trainium2 Bass kernel for nn_Discriminator (dense MLP + pairwise L1 diversity).

SPMD over 8 cores, data-parallel over the N=1024 rows (P=128 rows/core).
Dense layers run in bf16 (fp32 PSUM accumulate). The diversity term

    div[j,k] = sum_i exp( - sum_d |M[i,k,d] - M[j,k,d]| ),  M = h @ Wd + bd

uses |B - s| = 2*relu(B - s) - B + s per (k,d):
  - DVE tensor_scalar(subtract, max) in bf16 4x mode produces A_d tiles;
  - PE identity matmuls (bf16, 1 cy/row) accumulate the A_d over d into
    PSUM, one pair pre-added on DVE to balance engines;
  - a K=1 ones-row matmul adds the per-column -Sb/2 row (bf16, 1 cy/row);
  - ACT activation(Exp, scale=-2, bias=nss, accum_out=...) fuses the
    exponential with the row-sum over i.

Work split: core c handles kernel c for all eight 128-row J-blocks (units
0..7; its broadcast B tiles amortize 8x) plus kernels 8 and 9 for its own
block (units 8, 9). Collectives: an AllToAll of 11-row kernel shards
(10 M^T rows + a precomputed -Sb/2 row) hands each core its own kernel's
rows; an AllGather of rows 88..109 supplies kernels 8/9; a second AllToAll
returns div columns to their row owners.

The -Sb/2 rows ride the collective payload (computed once on the owner via
a grouped DVE reduce), so no per-unit fp32 ones-row or sbrow matmuls
remain. Self-terms are exact: nss is read back from the same bf16 -Sb/2
row (transposed slices for units 0..7; a per-core one-hot matmul for units
8/9), so exp(0)=1 lands exactly. All broadcasts are HWDGE DMA
row-broadcasts (Pool only issues the collectives), LN's rstd uses
Ln+Exp (one ACT table set, no reloads), and LeakyReLU is a single fused
scalar_tensor_tensor.
"""

import os
import sys

import numpy as np

sys.path.insert(0, "/opt/trn_rl_repo")

import concourse.bass as bass
import concourse.bacc as bacc
import concourse.tile as tile
from concourse import mybir
from concourse.bass_utils import run_bass_kernel_spmd

try:
    import ml_dtypes

    BF16_NP = ml_dtypes.bfloat16
except ImportError:  # pragma: no cover
    BF16_NP = None

F32 = mybir.dt.float32
BF16 = mybir.dt.bfloat16

N = 1024
NF = 512
HID = 256
NK = 10
KD = 10
MB = NK * KD  # 100
CAT = HID + NK  # 266
EPS = 1e-3
ALPHA = 0.3
NCORES = 8
P = N // NCORES  # 128 rows per core
ROWS11 = 11  # 10 M^T dims + 1 negsb row per kernel shard
MROWS = ROWS11 * NK  # 110

AF = mybir.ActivationFunctionType
ALU = mybir.AluOpType

# d-slices pre-added pairwise on DVE before the PE streams (per unit)
COMBINES = 1


def _chunks(total, size):
    out = []
    o = 0
    while o < total:
        out.append((o, min(size, total - o)))
        o += size
    return out


def build_program(stage="full"):
    nc = bacc.Bacc(
        "TRN2",
        target_bir_lowering=False,
        debug=False,
        num_devices=NCORES,
    )

    # ---- per-core external inputs ----
    xT = nc.dram_tensor("xT", [NF, P], BF16, kind="ExternalInput")
    W0 = nc.dram_tensor("W0", [NF, HID], BF16, kind="ExternalInput")
    b0c = nc.dram_tensor("b0c", [HID, 1], F32, kind="ExternalInput")
    Wd0 = nc.dram_tensor("Wd0", [HID, MB], BF16, kind="ExternalInput")
    bd0c = nc.dram_tensor("bd0c", [MB, 1], F32, kind="ExternalInput")
    beta0b = nc.dram_tensor("beta0b", [P, CAT], F32, kind="ExternalInput")
    W1 = nc.dram_tensor("W1", [CAT, HID], BF16, kind="ExternalInput")
    b1c = nc.dram_tensor("b1c", [HID, 1], F32, kind="ExternalInput")
    Wd1 = nc.dram_tensor("Wd1", [HID, MB], BF16, kind="ExternalInput")
    bd1c = nc.dram_tensor("bd1c", [MB, 1], F32, kind="ExternalInput")
    beta1b = nc.dram_tensor("beta1b", [P, CAT], F32, kind="ExternalInput")
    Wfb = nc.dram_tensor("Wfb", [P, CAT], F32, kind="ExternalInput")
    bfc = nc.dram_tensor("bfc", [P, 1], F32, kind="ExternalInput")
    # per-core one-hot [8, 1]: row c = 1 on core c (selects own J-block)
    selc = nc.dram_tensor("selc", [NCORES, 1], BF16, kind="ExternalInput")

    y_out = nc.dram_tensor("y", [P, 1], F32, kind="ExternalOutput")

    # ---- NEFF-embedded constants ----
    ident_f32 = nc.inline_tensor(np.eye(128, dtype=np.float32), name="ident_f32")
    ident_bf16 = nc.inline_tensor(
        np.eye(128).astype(BF16_NP), name="ident_bf16"
    )
    ones1_bf16 = nc.inline_tensor(
        np.ones((1, 128)).astype(BF16_NP), name="ones1_bf16"
    )

    with tile.TileContext(nc, num_cores=NCORES) as tc:
        dram = tc.alloc_tile_pool(name="dram", bufs=1, space="DRAM")
        m_loc = [dram.tile([MROWS, P], BF16, name=f"m_loc{b}") for b in range(2)]
        m_gath = [
            dram.tile(
                [NCORES, 2 * ROWS11, P], BF16,
                addr_space=("Local" if stage == "nocc" else "Shared"),
                name=f"m_gath{b}",
            )
            for b in range(2)
        ]
        # AllToAll of kernel shards 0..7 (11 rows each): every core receives
        # its own kernel's rows (incl. the negsb row) from all peers
        mtam_recv = [
            dram.tile([NCORES, ROWS11, P], BF16, name=f"mtam_r{b}")
            for b in range(2)
        ]
        # own kernel rows + kernels 8/9 rows assembled contiguously in DRAM
        # (broadcast DMAs need a contiguous DRAM source row)
        mtA_dram = [dram.tile([KD, N], BF16, name=f"mtA_d{b}") for b in range(2)]
        mt8_dram = [dram.tile([KD, N], BF16, name=f"mt8_d{b}") for b in range(2)]
        mt9_dram = [dram.tile([KD, N], BF16, name=f"mt9_d{b}") for b in range(2)]
        a2a_send = [dram.tile([NCORES, P], F32, name=f"a2a_s{b}") for b in range(2)]
        a2a_recv = [
            dram.tile([NCORES, P], F32, name=f"a2a_r{b}") for b in range(2)
        ]
        consts = tc.alloc_tile_pool(name="consts", bufs=1)
        acts = tc.alloc_tile_pool(name="acts", bufs=1)
        mtiles = tc.alloc_tile_pool(name="mtiles", bufs=2)
        bpool = tc.alloc_tile_pool(name="bpool", bufs=2)
        apool = tc.alloc_tile_pool(name="apool", bufs=6)
        cpool = tc.alloc_tile_pool(name="cpool", bufs=2)
        epool = tc.alloc_tile_pool(name="epool", bufs=2)
        small = tc.alloc_tile_pool(name="small", bufs=4)
        ps_small = tc.alloc_tile_pool(name="ps_small", bufs=2, space="PSUM")
        ps_l1 = tc.alloc_tile_pool(name="ps_l1", bufs=3, space="PSUM")

        # ---------- load constants ----------
        # startup-critical consts via HWDGE (sync); late-needed block-1/LN/
        # head weights ride the Pool queue (idle through the unit loop)
        def load(dram_t, shape, dtype=F32, name=None, late=False):
            t = consts.tile(shape, dtype, name=name)
            (nc.gpsimd if late else nc.sync).dma_start(out=t, in_=dram_t)
            return t

        xT_sb = [
            load(xT[o : o + sz, :], [sz, P], BF16, name=f"xT{i}")
            for i, (o, sz) in enumerate(_chunks(NF, 128))
        ]
        w0_sb = [
            load(W0[o : o + sz, :], [sz, HID], BF16, name=f"w0_{i}")
            for i, (o, sz) in enumerate(_chunks(NF, 128))
        ]
        idf = load(ident_f32[:, :], [128, 128], name="idf")
        idb = load(ident_bf16[:, :], [128, 128], BF16, name="idb")
        ones1 = load(ones1_bf16[:, :], [1, 128], BF16, name="ones1")
        selc_sb = load(selc[:, :], [NCORES, 1], BF16, name="selc")
        w1_sb = [
            load(W1[o : o + sz, :], [sz, HID], BF16, name=f"w1_{i}", late=True)
            for i, (o, sz) in enumerate(_chunks(CAT, 128))
        ]
        wd0_sb = [
            load(Wd0[o : o + sz, :], [sz, MB], BF16, name=f"wd0_{i}")
            for i, (o, sz) in enumerate(_chunks(HID, 128))
        ]
        wd1_sb = [
            load(Wd1[o : o + sz, :], [sz, MB], BF16, name=f"wd1_{i}", late=True)
            for i, (o, sz) in enumerate(_chunks(HID, 128))
        ]
        b0_sb = [
            load(b0c[o : o + sz, :], [sz, 1], name=f"b0_{i}")
            for i, (o, sz) in enumerate(_chunks(HID, 128))
        ]
        b1_sb = [
            load(b1c[o : o + sz, :], [sz, 1], name=f"b1_{i}", late=True)
            for i, (o, sz) in enumerate(_chunks(HID, 128))
        ]
        bd0_sb = load(bd0c[:, :], [MB, 1], name="bd0")
        bd1_sb = load(bd1c[:, :], [MB, 1], name="bd1", late=True)
        beta_sb = [
            load(beta0b[:, :], [P, CAT], name="beta0", late=True),
            load(beta1b[:, :], [P, CAT], name="beta1", late=True),
        ]
        wf_sb = load(Wfb[:, :], [P, CAT], name="wf", late=True)
        bf_sb = load(bfc[:, :], [P, 1], name="bf", late=True)

        eps_sb = consts.tile([P, 1], F32, name="eps")
        nc.vector.memset(eps_sb, EPS)

        def ap_of(t, ap, extra_off=0):
            return bass.AP(tensor=t.tensor, offset=t.offset + extra_off, ap=ap)

        # ---------- one block ----------
        def block(b, prevT, w_sb, b_sb, wd_sb, bd_sb):
            """prevT: list of (tile, psize) feature-major bf16 chunks.

            Returns [P, CAT] fp32 tile = LeakyReLU(LN(concat(h, div)) + beta).
            """
            # h^T = W^T @ prev + b   (feature-major, HID x P as 2 chunks)
            hT = []
            for mi, (mo, msz) in enumerate(_chunks(HID, 128)):
                ps = ps_small.tile([128, P], F32, tag="ps_small")
                for ki, (wt, (pt, psz)) in enumerate(zip(w_sb, prevT)):
                    nc.tensor.matmul(
                        ps[:msz, :],
                        wt[:psz, mo : mo + msz],
                        pt,
                        start=(ki == 0),
                        stop=(ki == len(w_sb) - 1),
                    )
                ht = acts.tile([msz, P], BF16, name=f"hT{b}_{mi}")
                nc.vector.tensor_scalar(
                    out=ht, in0=ps[:msz, :], scalar1=b_sb[mi], scalar2=None,
                    op0=ALU.add,
                )
                hT.append((ht, msz))

            # M^T = Wd^T @ h + bd   [100, 128] bf16
            ps_m = ps_small.tile([MB, P], F32, tag="ps_small")
            for ki, ((ht, _), wdt) in enumerate(zip(hT, wd_sb)):
                nc.tensor.matmul(
                    ps_m,
                    wdt,
                    ht,
                    start=(ki == 0),
                    stop=(ki == len(wd_sb) - 1),
                )
            mT = mtiles.tile([MB, P], BF16, tag="mT")
            nc.vector.tensor_scalar(
                out=mT, in0=ps_m, scalar1=bd_sb, scalar2=None, op0=ALU.add
            )

            # own M rows (row-major fp32) for units 8/9 scalars
            ps_t = ps_small.tile([128, MB], BF16, tag="ps_small")
            nc.tensor.transpose(ps_t[:, :], mT, idb[:MB, :MB])
            m_row = mtiles.tile([P, MB], F32, tag="m_row")
            nc.vector.tensor_copy(m_row, ps_t[:, :MB])

            # negsb rows: -Sb/2 per kernel for own rows, shipped with M^T.
            # grouped reduce [P, 10(k), 10(d)] -> [P, 10]
            sb_pk = mtiles.tile([P, NK], F32, tag="sb_pk")
            nc.vector.tensor_reduce(
                out=sb_pk,
                in_=ap_of(m_row, [m_row.ap[0], [KD, NK], [1, KD]]),
                axis=mybir.AxisListType.X,
                op=ALU.add,
            )
            ps_nb = ps_small.tile([128, P], F32, tag="ps_small")
            nc.tensor.transpose(ps_nb[:NK, :], sb_pk, idf)
            negsbT = mtiles.tile([NK, P], BF16, tag="negsbT")
            nc.vector.tensor_scalar(
                out=negsbT, in0=ps_nb[:NK, :], scalar1=-0.5, scalar2=None,
                op0=ALU.mult,
            )

            # ---- ship M^T + negsb rows (kernel-sharded layout) ----
            nc.sync.dma_start(
                out=ap_of(m_loc[b][:, :], [[ROWS11 * P, NK], [P, KD], [1, P]]),
                in_=mT,
            )
            nc.sync.dma_start(
                out=ap_of(m_loc[b][:, :], [[ROWS11 * P, NK], [1, P]],
                          extra_off=KD * P),
                in_=negsbT,
            )

            if stage == "nocc":
                nc.gpsimd.dma_start(
                    out=mtam_recv[b][:, :, :],
                    in_=m_loc[b][0 : NCORES * ROWS11, :],
                )
                for c in range(NCORES):
                    nc.sync.dma_start(
                        out=m_gath[b][c, :, :],
                        in_=m_loc[b][NCORES * ROWS11 : MROWS, :],
                    )
            else:
                nc.gpsimd.collective_compute(
                    "AllToAll",
                    ALU.bypass,
                    replica_groups=[list(range(NCORES))],
                    ins=[m_loc[b][0 : NCORES * ROWS11, :]],
                    outs=[mtam_recv[b][:, :, :]],
                )
                nc.gpsimd.collective_compute(
                    "AllGather",
                    ALU.bypass,
                    replica_groups=[list(range(NCORES))],
                    ins=[m_loc[b][NCORES * ROWS11 : MROWS, :]],
                    outs=[m_gath[b][:, :, :]],
                )

            # ---- assemble SBUF views + contiguous DRAM broadcast sources ----
            recv = mtam_recv[b][:, :, :]
            gath = m_gath[b][:, :, :]
            # own kernel: 10 M rows + negsb row across all 8 peer blocks
            mtA_sb = mtiles.tile([KD, N], BF16, tag="mtA_sb")
            nc.scalar.dma_start(
                out=mtA_sb,
                in_=ap_of(recv, [[P, KD], [ROWS11 * P, NCORES], [1, P]]),
            )
            negsbA = mtiles.tile([1, N], BF16, tag="negsbA")
            nc.scalar.dma_start(
                out=negsbA,
                in_=ap_of(recv, [[ROWS11 * P, NCORES], [1, P]],
                          extra_off=KD * P),
            )
            nc.sync.dma_start(
                out=mtA_dram[b][:, :],
                in_=ap_of(recv, [[P, KD], [ROWS11 * P, NCORES], [1, P]]),
            )
            # kernels 8/9 rows + negsb rows (+ partition-split negsb for nss)
            mt89_sb = []
            negsb89 = []
            nsb_split = []
            for k, mtk_dram in ((0, mt8_dram), (1, mt9_dram)):
                t = mtiles.tile([KD, N], BF16, tag=f"mt8{k}_sb")
                nc.scalar.dma_start(
                    out=t,
                    in_=ap_of(gath, [[P, KD], [2 * ROWS11 * P, NCORES], [1, P]],
                              extra_off=k * ROWS11 * P),
                )
                mt89_sb.append(t)
                r = mtiles.tile([1, N], BF16, tag=f"negsb8{k}")
                nc.scalar.dma_start(
                    out=r,
                    in_=ap_of(gath, [[2 * ROWS11 * P, NCORES], [1, P]],
                              extra_off=(k * ROWS11 + KD) * P),
                )
                negsb89.append(r)
                sp = mtiles.tile([NCORES, P], BF16, tag=f"nsbsp{k}")
                nc.scalar.dma_start(
                    out=sp,
                    in_=ap_of(gath, [[2 * ROWS11 * P, NCORES], [1, P]],
                              extra_off=(k * ROWS11 + KD) * P),
                )
                nsb_split.append(sp)
                nc.sync.dma_start(
                    out=mtk_dram[b][:, :],
                    in_=ap_of(gath, [[P, KD], [2 * ROWS11 * P, NCORES], [1, P]],
                              extra_off=k * ROWS11 * P),
                )

            # ---- broadcast B tiles: DMA row-broadcasts (HWDGE) ----
            btA = bpool.tile([P, KD * N], BF16, tag="btA")
            for d in range(KD):
                src = mtA_dram[b][d : d + 1, :]
                (nc.sync if d % 2 == 0 else nc.scalar).dma_start(
                    out=btA[:, d * N : (d + 1) * N],
                    in_=ap_of(src, [[0, P], [1, N]]),
                )
            bt89 = []
            for k, (mtk_dram, q) in enumerate(
                ((mt8_dram, nc.sync), (mt9_dram, nc.scalar))
            ):
                bt = bpool.tile([P, KD * N], BF16, tag=f"bt8{k}")
                src = mtk_dram[b][0:1, :]
                q.dma_start(out=bt, in_=ap_of(src, [[0, P], [1, KD * N]]))
                bt89.append(bt)

            # ---- per-unit scalars, all prepared upfront ----
            # scal[P, 10] per unit u<8: transpose of mtA_sb's u-th column block
            scalA = small.tile([P, NK * NCORES], F32, tag="scalA")
            nssA = small.tile([P, NCORES + 2], F32, tag="nssA")
            for u in range(NCORES):
                ps_sc = ps_small.tile([128, KD], BF16, tag="ps_small")
                nc.tensor.transpose(
                    ps_sc[:, :KD],
                    mtA_sb[:, u * P : (u + 1) * P],
                    idb[:KD, :KD],
                )
                nc.vector.tensor_copy(
                    scalA[:, u * KD : (u + 1) * KD], ps_sc[:, :KD]
                )
                # nss = +2 * negsbA[own J-block u columns]  (exact self-term)
                ps_ns = ps_small.tile([128, 1], BF16, tag="ps_small")
                nc.tensor.transpose(
                    ps_ns[:, 0:1],
                    negsbA[:, u * P : (u + 1) * P],
                    idb[:1, :1],
                )
                nc.vector.tensor_scalar(
                    out=nssA[:, u : u + 1], in0=ps_ns[:, 0:1],
                    scalar1=2.0, scalar2=None, op0=ALU.mult,
                )
            # units 8/9: nss = 2 * negsb89[own core's block] via one-hot matmul
            for k in range(2):
                ps_n8 = ps_small.tile([128, 1], F32, tag="ps_small")
                nc.tensor.matmul(
                    ps_n8[:, 0:1], nsb_split[k], selc_sb, start=True, stop=True
                )
                nc.vector.tensor_scalar(
                    out=nssA[:, NCORES + k : NCORES + k + 1], in0=ps_n8[:, 0:1],
                    scalar1=2.0, scalar2=None, op0=ALU.mult,
                )

            cat = acts.tile([P, CAT], F32, name=f"cat{b}")
            divsend = acts.tile([P, NCORES], F32, name=f"divsend{b}")

            # ---- unit loop ----
            for u in range(NK):
                if u < NCORES:
                    bt = btA
                    negsb = negsbA

                    def scal_col(d, _u=u):
                        return scalA[:, _u * KD + d : _u * KD + d + 1]

                    nss = nssA[:, u : u + 1]
                    accum_dst = divsend[:, u : u + 1]
                else:
                    k = u - NCORES
                    bt = bt89[k]
                    negsb = negsb89[k]

                    def scal_col(d, _k=k):
                        return m_row[:, (NCORES + _k) * KD + d :
                                     (NCORES + _k) * KD + d + 1]

                    nss = nssA[:, u : u + 1]
                    accum_dst = cat[:, HID + u : HID + u + 1]

                def relu_d(d):
                    at = apool.tile([P, N], BF16, tag="at")
                    nc.vector.tensor_scalar(
                        out=at,
                        in0=bt[:, d * N : (d + 1) * N],
                        scalar1=scal_col(d),
                        scalar2=0.0,
                        op0=ALU.subtract,
                        op1=ALU.max,
                    )
                    return at

                psl = ps_l1.tile([P, N], F32, tag="psl")

                def stream(at, first):
                    for ho, hsz in _chunks(N, 512):
                        nc.tensor.matmul(
                            psl[:, ho : ho + hsz],
                            idb,
                            at[:, ho : ho + hsz],
                            start=first,
                            stop=False,
                        )

                n_direct = KD - 2 * COMBINES
                for d in range(n_direct):
                    stream(relu_d(d), d == 0)
                for ci in range(COMBINES):
                    lo = n_direct + 2 * ci
                    a0, a1 = relu_d(lo), relu_d(lo + 1)
                    comb = cpool.tile([P, N], BF16, tag="comb")
                    nc.vector.tensor_add(comb, a0, a1)
                    stream(comb, False)
                for ho, hsz in _chunks(N, 512):
                    nc.tensor.matmul(
                        psl[:, ho : ho + hsz],
                        ones1,
                        negsb[:, ho : ho + hsz],
                        start=False,
                        stop=True,
                    )
                escr = epool.tile([P, N], BF16, tag="escr")
                nc.scalar.activation(
                    escr, psl, AF.Exp, bias=nss, scale=-2.0,
                    accum_out=accum_dst,
                )

            # ---- exchange div columns (AllToAll) ----
            ps_ds = ps_small.tile([128, P], F32, tag="ps_small")
            nc.tensor.transpose(ps_ds[:NCORES, :], divsend, idf)
            dsend_sb = small.tile([NCORES, P], F32, tag="dsend")
            nc.vector.tensor_copy(dsend_sb, ps_ds[:NCORES, :])
            nc.sync.dma_start(out=a2a_send[b][:, :], in_=dsend_sb)
            if stage == "nocc":
                nc.gpsimd.dma_start(
                    out=a2a_recv[b][:, :], in_=a2a_send[b][:, :]
                )
            else:
                nc.gpsimd.collective_compute(
                    "AllToAll",
                    ALU.bypass,
                    replica_groups=[list(range(NCORES))],
                    ins=[a2a_send[b][:, :]],
                    outs=[a2a_recv[b][:, :]],
                )
            drecv_sb = small.tile([NCORES, P], F32, tag="drecv")
            nc.scalar.dma_start(out=drecv_sb, in_=a2a_recv[b][:, :])
            ps_dr = ps_small.tile([128, NCORES], F32, tag="ps_small")
            nc.tensor.transpose(
                ps_dr[:, :NCORES], drecv_sb, idf[:NCORES, :NCORES]
            )
            nc.vector.tensor_copy(
                cat[:, HID : HID + NCORES], ps_dr[:, :NCORES]
            )

            # h rows into cat[:, :256] via PE transposes of hT (bf16)
            for mi, (ht, msz) in enumerate(hT):
                ps_t2 = ps_small.tile([128, P], BF16, tag="ps_small")
                nc.tensor.transpose(ps_t2[:, :msz], ht, idb[:msz, :msz])
                nc.vector.tensor_copy(
                    cat[:, mi * 128 : mi * 128 + msz], ps_t2[:, :msz]
                )

            # LayerNorm (center+scale, beta only); rstd = exp(-0.5*ln(var+eps))
            # keeps ACT inside the exp/relu/copy table set (no reloads)
            stats = small.tile([P, 6], F32, tag="stats")
            nc.vector.bn_stats(out=stats, in_=cat)
            mv = small.tile([P, 2], F32, tag="mv")
            nc.vector.bn_aggr(out=mv, in_=stats)
            lnv = small.tile([P, 1], F32, tag="lnv")
            nc.scalar.activation(lnv, mv[:, 1:2], AF.Ln, bias=eps_sb, scale=1.0)
            rstd = small.tile([P, 1], F32, tag="rstd")
            nc.scalar.activation(rstd, lnv, AF.Exp, bias=0.0, scale=-0.5)
            catn = acts.tile([P, CAT], F32, name=f"catn{b}")
            nc.vector.tensor_scalar(
                out=catn,
                in0=cat,
                scalar1=mv[:, 0:1],
                scalar2=rstd,
                op0=ALU.subtract,
                op1=ALU.mult,
            )
            nc.vector.tensor_add(catn, catn, beta_sb[b])
            # leaky relu in one fused op: max(0.3*x, x)
            hout = acts.tile([P, CAT], F32, name=f"hout{b}")
            nc.vector.scalar_tensor_tensor(
                out=hout, in0=catn, scalar=ALPHA, in1=catn,
                op0=ALU.mult, op1=ALU.max,
            )
            return hout

        # ---------- block 0 ----------
        prev0 = [(t, 128) for t in xT_sb]
        h1 = block(0, prev0, w0_sb, b0_sb, wd0_sb, bd0_sb)

        # transpose h1 -> feature-major bf16 chunks for block 1
        h1T = []
        for ci, (co, csz) in enumerate(_chunks(CAT, 128)):
            ps_t = ps_small.tile([128, P], F32, tag="ps_small")
            nc.tensor.transpose(ps_t[:csz, :], h1[:, co : co + csz], idf)
            ht = acts.tile([csz, P], BF16, name=f"h1T_{ci}")
            nc.vector.tensor_copy(ht, ps_t[:csz, :])
            h1T.append((ht, csz))

        # ---------- block 1 ----------
        h2 = block(1, h1T, w1_sb, b1_sb, wd1_sb, bd1_sb)

        # ---------- critic head: y = h2 @ Wf + bf ----------
        hw = acts.tile([P, CAT], F32, name="hw")
        yacc = small.tile([P, 1], F32, tag="yacc")
        nc.vector.tensor_mul(hw, h2, wf_sb)
        nc.vector.tensor_reduce(
            out=yacc, in_=hw, axis=mybir.AxisListType.X, op=ALU.add
        )
        ysb = small.tile([P, 1], F32, tag="ysb")
        nc.vector.tensor_scalar(
            out=ysb, in0=yacc, scalar1=bf_sb, scalar2=None, op0=ALU.add
        )
        nc.sync.dma_start(out=y_out[:, :], in_=ysb)

        ps_l1.release()
        ps_small.release()
        small.release()
        epool.release()
        cpool.release()
        apool.release()
        bpool.release()
        mtiles.release()
        acts.release()
        consts.release()
        dram.release()

    nc.compile()
    return nc


_NC_CACHE = {}


def _get_nc():
    stage = os.environ.get("KERNEL_STAGE", "full")
    if stage not in _NC_CACHE:
        _NC_CACHE[stage] = build_program(stage)
    return _NC_CACHE[stage]


def _make_in_maps(inputs):
    if BF16_NP is None:
        raise RuntimeError("ml_dtypes required for bf16 inputs")
    f = lambda a: np.ascontiguousarray(np.asarray(a, dtype=np.float32))
    bf = lambda a: np.ascontiguousarray(np.asarray(a, dtype=np.float32)).astype(
        BF16_NP
    )
    x = f(inputs["x"])
    shared = {
        "W0": bf(inputs["W0"]),
        "b0c": f(inputs["b0"]).reshape(HID, 1),
        "Wd0": bf(inputs["Wd0"]),
        "bd0c": f(inputs["bd0"]).reshape(MB, 1),
        "beta0b": np.ascontiguousarray(
            np.broadcast_to(f(inputs["beta0"]), (P, CAT))
        ),
        "W1": bf(inputs["W1"]),
        "b1c": f(inputs["b1"]).reshape(HID, 1),
        "Wd1": bf(inputs["Wd1"]),
        "bd1c": f(inputs["bd1"]).reshape(MB, 1),
        "beta1b": np.ascontiguousarray(
            np.broadcast_to(f(inputs["beta1"]), (P, CAT))
        ),
        "Wfb": np.ascontiguousarray(
            np.broadcast_to(f(inputs["Wf"]).reshape(1, CAT), (P, CAT))
        ),
        "bfc": np.full((P, 1), float(np.asarray(inputs["bf"]).reshape(-1)[0]),
                       dtype=np.float32),
    }
    in_maps = []
    for c in range(NCORES):
        m = dict(shared)
        m["xT"] = np.ascontiguousarray(x[c * P : (c + 1) * P, :].T).astype(
            BF16_NP
        )
        sel = np.zeros((NCORES, 1), dtype=np.float32)
        sel[c, 0] = 1.0
        m["selc"] = sel.astype(BF16_NP)
        in_maps.append(m)
    return in_maps


def run(inputs, **kw):
    nc = _get_nc()
    in_maps = _make_in_maps(inputs)
    res = run_bass_kernel_spmd(nc, in_maps, list(range(NCORES)), **kw)
    y = np.concatenate([res.results[c]["y"] for c in range(NCORES)], axis=0)
    return y.astype(np.float32), res


def kernel(**inputs) -> np.ndarray:
    y, _ = run(inputs)
    return y


# revision 73
# speedup vs baseline: 1.7806x; 1.7611x over previous
"""Trainium2 Bass kernel for nn_Discriminator (dense MLP + pairwise L1 diversity).

SPMD over 8 cores, data-parallel over the N=1024 rows (P=128 rows/core).
Dense layers run in bf16 (fp32 PSUM accumulate). The diversity term

    div[j,k] = sum_i exp( - sum_d |M[i,k,d] - M[j,k,d]| ),  M = h @ Wd + bd

uses |B - s| = 2*relu(B - s) - B + s per (k,d):
  - DVE tensor_scalar(subtract, max) in bf16 4x mode produces A_d tiles
    (one d per unit offloaded to ACT Relu where the span is wide);
  - PE identity matmuls (bf16, 1 cy/row) accumulate the A_d over d into
    PSUM, one pair pre-added on DVE to balance engines;
  - a K=1 ones-row matmul adds the per-column -Sb/2 row (bf16, 1 cy/row);
  - ACT activation(Exp, scale=-2, bias=-Ss, accum_out=...) fuses the
    exponential with the row-sum over i.

Work split: core c handles kernel c for all eight 128-row J-blocks (units
0..7) plus kernels 8/9 for its own block (units 8, 9). Since the own
kernel's full 1024x1024 l1 matrix is local and symmetric, unit u computes
only columns [128u, 1024) (diag block included); the missing i < 128u
contributions are PE column-sums of earlier units' exp tiles, accumulated
in a PSUM row and transposed back into the div columns (~44% less
relu/stream/exp work for units 0..7).

Collectives: an AllToAll of 11-row kernel shards (a -Sb/2 row at
partition 0 + 10 M^T rows, computed on the owner via one block-diagonal
-0.5-ones matmul) hands each core its own kernel's rows; an AllGather of
rows 88..109 supplies kernels 8/9; a second AllToAll returns div columns
to their row owners, fired right after unit 7 so it overlaps units 8/9.

Per-unit scalars come from one 11-row PE transpose per J-block into a
stride-12 PSUM tile (negsb at even offsets; odd-offset bf16 PSUM reads
fail the hardware ISA check) drained by two full-tile copies; nss is read
from the same bf16 -Sb/2 values the ones-rows use, so exp(0)=1 is exact.
B tiles are per-d HWDGE(sync)/SWDGE(pool) DMA row-broadcasts issued in
consumption order (d0 straight from the scattered recv blocks); kernels
8/9 ride two mega-broadcasts behind them. The ACT queue carries only
activations (DMAs on it stall the exps behind their issue latency), and
ACT stays on the single exp/relu/copy table set: LN's rstd is a DVE-only
fast inverse sqrt (bit-trick seed + 2 Newton steps), and LeakyReLU is a
fused scalar_tensor_tensor. M travels in bf16; PSUM/LN stay fp32
(rel err ~5e-3).
"""

import os
import sys

import numpy as np

sys.path.insert(0, "/opt/trn_rl_repo")

import concourse.bass as bass
import concourse.bacc as bacc
import concourse.tile as tile
from concourse import mybir
from concourse.bass_utils import run_bass_kernel_spmd

try:
    import ml_dtypes

    BF16_NP = ml_dtypes.bfloat16
except ImportError:  # pragma: no cover
    BF16_NP = None

F32 = mybir.dt.float32
BF16 = mybir.dt.bfloat16

N = 1024
NF = 512
HID = 256
NK = 10
KD = 10
MB = NK * KD  # 100
CAT = HID + NK  # 266
EPS = 1e-3
ALPHA = 0.3
NCORES = 8
P = N // NCORES  # 128 rows per core
ROWS11 = 11  # 10 M^T dims + 1 negsb row per kernel shard
MROWS = ROWS11 * NK  # 110

AF = mybir.ActivationFunctionType
ALU = mybir.AluOpType

# d-slices pre-added pairwise on DVE before the PE streams (per unit)
COMBINES = 1
POOL_RELU = False
ACT_W = 640
N_WARM = 24


def _chunks(total, size):
    out = []
    o = 0
    while o < total:
        out.append((o, min(size, total - o)))
        o += size
    return out


def build_program(stage="full"):
    nc = bacc.Bacc(
        "TRN2",
        target_bir_lowering=False,
        debug=False,
        num_devices=NCORES,
    )

    # ---- per-core external inputs ----
    xT = nc.dram_tensor("xT", [NF, P], BF16, kind="ExternalInput")
    W0 = nc.dram_tensor("W0", [NF, HID], BF16, kind="ExternalInput")
    b0c = nc.dram_tensor("b0c", [HID, 1], F32, kind="ExternalInput")
    Wd0 = nc.dram_tensor("Wd0", [HID, MB], BF16, kind="ExternalInput")
    bd0c = nc.dram_tensor("bd0c", [MB, 1], F32, kind="ExternalInput")
    beta0b = nc.dram_tensor("beta0b", [P, CAT], F32, kind="ExternalInput")
    W1 = nc.dram_tensor("W1", [CAT, HID], BF16, kind="ExternalInput")
    b1c = nc.dram_tensor("b1c", [HID, 1], F32, kind="ExternalInput")
    Wd1 = nc.dram_tensor("Wd1", [HID, MB], BF16, kind="ExternalInput")
    bd1c = nc.dram_tensor("bd1c", [MB, 1], F32, kind="ExternalInput")
    beta1b = nc.dram_tensor("beta1b", [P, CAT], F32, kind="ExternalInput")
    Wfb = nc.dram_tensor("Wfb", [P, CAT], F32, kind="ExternalInput")
    bfc = nc.dram_tensor("bfc", [P, 1], F32, kind="ExternalInput")
    y_out = nc.dram_tensor("y", [P, 1], F32, kind="ExternalOutput")

    # ---- NEFF-embedded constants ----
    ident_f32 = nc.inline_tensor(np.eye(128, dtype=np.float32), name="ident_f32")
    ident_bf16 = nc.inline_tensor(
        np.eye(128).astype(BF16_NP), name="ident_bf16"
    )
    ones1_bf16 = nc.inline_tensor(
        np.ones((1, 128)).astype(BF16_NP), name="ones1_bf16"
    )
    onesc_bf16 = nc.inline_tensor(
        np.ones((128, 1)).astype(BF16_NP), name="onesc_bf16"
    )
    _kblk = np.zeros((MB, NK))
    for _k in range(NK):
        _kblk[_k * KD:(_k + 1) * KD, _k] = -0.5
    kblk_bf16 = nc.inline_tensor(_kblk.astype(BF16_NP), name="kblk_bf16")

    with tile.TileContext(nc, num_cores=NCORES) as tc:
        dram = tc.alloc_tile_pool(name="dram", bufs=1, space="DRAM")
        m_loc = [dram.tile([MROWS, P], BF16, name=f"m_loc{b}") for b in range(2)]
        m_gath = [
            dram.tile(
                [NCORES, 2 * ROWS11, P], BF16,
                addr_space=("Local" if stage == "nocc" else "Shared"),
                name=f"m_gath{b}",
            )
            for b in range(2)
        ]
        # AllToAll of kernel shards 0..7 (11 rows each): every core receives
        # its own kernel's rows (incl. the negsb row) from all peers
        mtam_recv = [
            dram.tile([NCORES, ROWS11, P], BF16, name=f"mtam_r{b}")
            for b in range(2)
        ]
        # own kernel rows + kernels 8/9 rows assembled contiguously in DRAM
        # (broadcast DMAs need a contiguous DRAM source row)
        mtA_dram = [dram.tile([KD, N], BF16, name=f"mtA_d{b}") for b in range(2)]
        mt8_dram = [dram.tile([KD, N], BF16, name=f"mt8_d{b}") for b in range(2)]
        mt9_dram = [dram.tile([KD, N], BF16, name=f"mt9_d{b}") for b in range(2)]
        a2a_send = [dram.tile([NCORES, P], F32, name=f"a2a_s{b}") for b in range(2)]
        a2a_recv = [
            dram.tile([NCORES, P], F32, name=f"a2a_r{b}") for b in range(2)
        ]
        consts = tc.alloc_tile_pool(name="consts", bufs=1)
        acts = tc.alloc_tile_pool(name="acts", bufs=1)
        mtiles = tc.alloc_tile_pool(name="mtiles", bufs=2)
        bpool = tc.alloc_tile_pool(name="bpool", bufs=2)
        apool = tc.alloc_tile_pool(name="apool", bufs=10)
        cpool = tc.alloc_tile_pool(name="cpool", bufs=2)
        epool = tc.alloc_tile_pool(name="epool", bufs=10)
        small = tc.alloc_tile_pool(name="small", bufs=2)
        ps_small = tc.alloc_tile_pool(name="ps_small", bufs=1, space="PSUM")
        ps_prep = tc.alloc_tile_pool(name="ps_prep", bufs=1, space="PSUM")
        ps_col = tc.alloc_tile_pool(name="ps_col", bufs=1, space="PSUM")
        ps_l1 = tc.alloc_tile_pool(name="ps_l1", bufs=2, space="PSUM")

        def ap_of(t, ap, extra_off=0):
            return bass.AP(tensor=t.tensor, offset=t.offset + extra_off, ap=ap)

        # ---------- load constants ----------
        # startup-critical consts via HWDGE (sync), each k-chunked weight
        # merged into a single [128, n*cols] tile via one strided DMA;
        # late-needed block-1/LN/head weights ride the Pool queue
        def load(dram_t, shape, dtype=F32, name=None, late=False, src_ap=None):
            t = consts.tile(shape, dtype, name=name)
            # late consts ride the Pool queue, but are emitted only after
            # block 0's collectives (emit_late_consts) so they don't block
            # the m-chain; ones1/selc ride the scalar queue ahead of its
            # first real work
            q = nc.gpsimd if late else nc.sync
            q.dma_start(out=t, in_=(src_ap if src_ap is not None else dram_t))
            return t

        def load_chunked(dram_t, rows, cols, nch, dtype=BF16, name=None,
                         late=False):
            # [nch*128, cols] dram -> [128, nch*cols] sbuf, one DMA
            t = load(
                dram_t, [128, nch * cols], dtype, name=name, late=late,
                src_ap=ap_of(dram_t[:, :],
                             [[cols, 128], [128 * cols, nch], [1, cols]]),
            )
            return [t[:, k * cols : (k + 1) * cols] for k in range(nch)]

        xT_sb = load_chunked(xT, NF, P, 4, name="xTall")
        w0_sb = load_chunked(W0, NF, HID, 4, name="w0all")
        b0_t = load(b0c[:, :], [128, 2], name="b0all",
                    src_ap=ap_of(b0c[:, :], [[1, 128], [128, 2]]))
        b0_sb = [b0_t[:, 0:1], b0_t[:, 1:2]]
        wd0_sb = load_chunked(Wd0, HID, MB, 2, name="wd0all")
        bd0_sb = load(bd0c[:, :], [MB, 1], name="bd0")
        kblk = load(kblk_bf16[:, :], [MB, NK], BF16, name="kblk")
        idb = load(ident_bf16[:, :], [128, 128], BF16, name="idb")
        idf = load(ident_f32[:, :], [128, 128], name="idf")
        ones1 = consts.tile([1, 128], BF16, name="ones1")
        nc.scalar.dma_start(out=ones1, in_=ones1_bf16[:, :])
        onesc = consts.tile([128, 1], BF16, name="onesc")
        nc.scalar.dma_start(out=onesc, in_=onesc_bf16[:, :])

        late_sb = {}

        def emit_late_consts():
            w1_sb = load_chunked(W1, 256, HID, 2, name="w1all", late=True)
            w1c3 = load(W1[256:CAT, :], [10, HID], BF16, name="w1c3",
                        late=True)
            late_sb["w1"] = w1_sb + [w1c3[:, :]]
            late_sb["wd1"] = load_chunked(Wd1, HID, MB, 2, name="wd1all",
                                          late=True)
            b1_t = load(b1c[:, :], [128, 2], name="b1all", late=True,
                        src_ap=ap_of(b1c[:, :], [[1, 128], [128, 2]]))
            late_sb["b1"] = [b1_t[:, 0:1], b1_t[:, 1:2]]
            late_sb["bd1"] = load(bd1c[:, :], [MB, 1], name="bd1", late=True)
            late_sb["beta"] = [
                load(beta0b[:, :], [P, CAT], name="beta0", late=True),
                load(beta1b[:, :], [P, CAT], name="beta1", late=True),
            ]
            late_sb["wf"] = load(Wfb[:, :], [P, CAT], name="wf", late=True)
            late_sb["bf"] = load(bfc[:, :], [P, 1], name="bf", late=True)

        # ---------- one block ----------
        def block(b, prevT, w_sb, b_sb, wd_sb, bd_sb, after_ship=None):
            """prevT: list of (tile, psize) feature-major bf16 chunks.

            Returns [P, CAT] fp32 tile = LeakyReLU(LN(concat(h, div)) + beta).
            """
            # h^T = W^T @ prev + b   (feature-major, HID x P as 2 chunks)
            hT = []
            for mi, (mo, msz) in enumerate(_chunks(HID, 128)):
                ps = ps_small.tile([128, P], F32, tag="ps_small")
                for ki, (wt, (pt, psz)) in enumerate(zip(w_sb, prevT)):
                    nc.tensor.matmul(
                        ps[:msz, :],
                        wt[:psz, mo : mo + msz],
                        pt,
                        start=(ki == 0),
                        stop=(ki == len(w_sb) - 1),
                    )
                ht = acts.tile([msz, P], BF16, name=f"hT{b}_{mi}")
                nc.vector.tensor_scalar(
                    out=ht, in0=ps[:msz, :], scalar1=b_sb[mi], scalar2=None,
                    op0=ALU.add,
                )
                hT.append((ht, msz))

            # M^T = Wd^T @ h + bd   [100, 128] bf16
            ps_m = ps_small.tile([MB, P], F32, tag="ps_small")
            for ki, ((ht, _), wdt) in enumerate(zip(hT, wd_sb)):
                nc.tensor.matmul(
                    ps_m,
                    wdt,
                    ht,
                    start=(ki == 0),
                    stop=(ki == len(wd_sb) - 1),
                )
            mT = mtiles.tile([MB, P], BF16, tag="mT")
            nc.vector.tensor_scalar(
                out=mT, in0=ps_m, scalar1=bd_sb, scalar2=None, op0=ALU.add
            )

            # negsb rows: -Sb/2 per kernel for own rows (shipped with M^T);
            # one block-diagonal ones matmul straight off mT keeps this on
            # the m_loc critical path w/o waiting for the m_row transpose
            ps_nb = ps_small.tile([NK, P], F32, tag="ps_small")
            nc.tensor.matmul(ps_nb, kblk, mT, start=True, stop=True)
            negsbT = mtiles.tile([NK, P], BF16, tag="negsbT")
            nc.vector.tensor_copy(negsbT, ps_nb)

            # own M rows (row-major fp32) for units 8/9 scalars
            ps_t = ps_small.tile([128, MB], BF16, tag="ps_small")
            nc.tensor.transpose(ps_t[:, :], mT, idb[:MB, :MB])
            m_row = mtiles.tile([P, MB], F32, tag="m_row")
            nc.vector.tensor_copy(m_row, ps_t[:, :MB])

            # keep the PE pipeline warm through the collective wait: the
            # pstate model drops to cold after any idle, which would tax the
            # first units' streams; these dummies are never read
            if N_WARM:
                ps_w = ps_l1.tile([P, N], F32, tag="psl")
                for _ in range(N_WARM):
                    nc.tensor.matmul(
                        ps_w[:, 0:128], idb, idb,
                        start=True, stop=True, skip_group_check=True,
                    )

            # ---- ship M^T + negsb rows (kernel-sharded layout, negsb
            # first in each 11-row shard so receivers get it at partition 0)
            nc.sync.dma_start(
                out=ap_of(m_loc[b][:, :], [[ROWS11 * P, NK], [P, KD], [1, P]],
                          extra_off=P),
                in_=mT,
            )
            nc.sync.dma_start(
                out=ap_of(m_loc[b][:, :], [[ROWS11 * P, NK], [1, P]]),
                in_=negsbT,
            )

            if stage == "nocc":
                nc.gpsimd.dma_start(
                    out=mtam_recv[b][:, :, :],
                    in_=m_loc[b][0 : NCORES * ROWS11, :],
                )
            else:
                nc.gpsimd.collective_compute(
                    "AllToAll",
                    ALU.bypass,
                    replica_groups=[list(range(NCORES))],
                    ins=[m_loc[b][0 : NCORES * ROWS11, :]],
                    outs=[mtam_recv[b][:, :, :]],
                )

            def emit_gather():
                if stage == "nocc":
                    src89 = m_loc[b][NCORES * ROWS11 : MROWS, :]
                    nc.gpsimd.dma_start(
                        out=m_gath[b][:, :, :],
                        in_=ap_of(src89,
                                  [[0, NCORES], [P, 2 * ROWS11], [1, P]]),
                    )
                else:
                    nc.gpsimd.collective_compute(
                        "AllGather",
                        ALU.bypass,
                        replica_groups=[list(range(NCORES))],
                        ins=[m_loc[b][NCORES * ROWS11 : MROWS, :]],
                        outs=[m_gath[b][:, :, :]],
                    )

            # ---- assemble SBUF views + contiguous DRAM broadcast sources ----
            recv = mtam_recv[b][:, :, :]
            gath = m_gath[b][:, :, :]
            # own kernel: one [11, N] tile = negsb row 0 + 10 M rows
            mtA11 = mtiles.tile([ROWS11, N], BF16, tag="mtA11")
            nc.gpsimd.dma_start(
                out=mtA11,
                in_=ap_of(recv, [[P, ROWS11], [ROWS11 * P, NCORES], [1, P]]),
            )

            # ---- broadcast B tiles: HWDGE(sync)/SWDGE(pool) row-broadcasts,
            # per-d slices in consumption order; d=0 broadcasts straight from
            # the scattered recv blocks (skips the mtA_dram hop + its sem)
            btA = bpool.tile([P, KD * N], BF16, tag="btA")
            nc.sync.dma_start(
                out=mtA_dram[b][:, :],
                in_=ap_of(recv, [[P, KD], [ROWS11 * P, NCORES], [1, P]],
                          extra_off=P),
            )
            nc.sync.dma_start(
                out=btA[:, 0:N],
                in_=ap_of(recv, [[0, P], [ROWS11 * P, NCORES], [1, P]],
                          extra_off=P),
            )
            for d in range(1, KD):
                src = mtA_dram[b][d : d + 1, :]
                (nc.sync if d % 2 == 0 else nc.gpsimd).dma_start(
                    out=btA[:, d * N : (d + 1) * N],
                    in_=ap_of(src, [[0, P], [1, N]]),
                )
            negsbA = mtA11[0:1, :]
            emit_gather()
            # kernels 8/9: DRAM broadcast sources (M rows only) + one
            # [11, N] SBUF tile each (negsb at partition 0 + 10 M rows)
            negsb89 = []
            for k, mtk_dram in ((0, mt8_dram), (1, mt9_dram)):
                nc.gpsimd.dma_start(
                    out=mtk_dram[b][:, :],
                    in_=ap_of(gath,
                              [[P, KD], [2 * ROWS11 * P, NCORES], [1, P]],
                              extra_off=(k * ROWS11 + 1) * P),
                )
                t = mtiles.tile([1, N], BF16, tag=f"nsb8{k}")
                nc.gpsimd.dma_start(
                    out=t,
                    in_=ap_of(gath, [[2 * ROWS11 * P, NCORES], [1, P]],
                              extra_off=k * ROWS11 * P),
                )
                negsb89.append(t)
            bt89 = []
            for k, mtk_dram in ((0, mt8_dram), (1, mt9_dram)):
                bt = bpool.tile([P, KD * N], BF16, tag=f"bt8{k}")
                s = mtk_dram[b][0:1, :]
                nc.gpsimd.dma_start(out=bt, in_=ap_of(s, [[0, P], [1, KD * N]]))
                bt89.append(bt)
            if after_ship is not None:
                after_ship()

            # ---- per-unit scalars, one fused PSUM tile ----
            # one 11-row transpose per unit yields the negsb slice (col 0,
            # the nss source) AND the 10 scal values (cols 1..10) at once;
            # cols 88..97 hold negsbT transposed (own rows, all kernels)
            # stride-12 unit layout keeps each bf16 transpose output at an
            # even column (PSUM matmul writes must be 4-byte aligned)
            ps_pp = ps_prep.tile([128, 108], BF16, tag="ps_prep")
            for u in range(NCORES):
                nc.tensor.transpose(
                    ps_pp[:, u * 12 : u * 12 + ROWS11],
                    mtA11[:, u * P : (u + 1) * P],
                    idb[:ROWS11, :ROWS11],
                )
            nc.tensor.transpose(
                ps_pp[:, 96:106], negsbT, idb[:KD, :KD]
            )
            # full-tile aligned PSUM->SBUF copies (odd bf16 PSUM offsets
            # fail the ISA check); per-unit scalars are then SBUF slices
            ppAll = small.tile([P, 108], F32, tag="ppAll")
            nc.vector.tensor_copy(ppAll, ps_pp)
            nppAll = small.tile([P, 108], F32, tag="nppAll")
            nc.vector.tensor_scalar(
                out=nppAll, in0=ps_pp,
                scalar1=-1.0, scalar2=None, op0=ALU.mult,
            )
            nscal89 = small.tile([P, 2 * KD], F32, tag="nscal89")
            nc.vector.tensor_scalar(
                out=nscal89, in0=m_row[:, NCORES * KD : MB],
                scalar1=-1.0, scalar2=None, op0=ALU.mult,
            )
            nssA = small.tile([P, NCORES], F32, tag="nssA")
            nc.vector.tensor_scalar(
                out=nssA,
                in0=bass.AP(tensor=ppAll.tensor, offset=ppAll.offset,
                            ap=[ppAll.ap[0], [12, NCORES]]),
                scalar1=2.0, scalar2=None, op0=ALU.mult,
            )
            # units 8/9: nss = -Ss(own rows) = 2 * (-Sb/2) from negsbT
            nss89 = small.tile([P, 2], F32, tag="nss89")
            nc.vector.tensor_scalar(
                out=nss89, in0=ppAll[:, 104:106],
                scalar1=2.0, scalar2=None, op0=ALU.mult,
            )

            cat = acts.tile([P, CAT], F32, name=f"cat{b}")
            divsend = acts.tile([P, NCORES], F32, name=f"divsend{b}")

            # h rows into cat[:, :256] via PE transposes of hT (bf16);
            # emitted early — fills PE/DVE during the m-chain wait
            for mi, (ht, msz) in enumerate(hT):
                ps_t2 = ps_small.tile([128, P], BF16, tag="ps_small")
                nc.tensor.transpose(ps_t2[:, :msz], ht, idb[:msz, :msz])
                nc.vector.tensor_copy(
                    cat[:, mi * 128 : mi * 128 + msz], ps_t2[:, :msz]
                )

            # ---- unit loop ----
            # Symmetry: for the own kernel the full 1024x1024 l1 matrix
            # lives on this core, so unit u computes only columns
            # [128u, 1024) (diag block included). The missing i < 128u
            # contributions are column sums of earlier units' exp tiles,
            # accumulated into a PSUM row and transposed into divsend.
            def span_chunks(off):
                if off < 512:
                    return [(off, 512 - off), (512, 512)]
                return [(off, N - off)]

            escr_tiles = {}

            def unit_ctx(u):
                ctx = {}
                if u < NCORES:
                    ctx["off"] = u * P
                    ctx["bt"] = btA
                    ctx["negsb"] = negsbA
                    ctx["scal"] = lambda d, _u=u: ppAll[
                        :, _u * 12 + 1 + d : _u * 12 + 2 + d]
                    ctx["nscal"] = lambda d, _u=u: nppAll[
                        :, _u * 12 + 1 + d : _u * 12 + 2 + d]
                    ctx["accum"] = divsend[:, u : u + 1]
                    ctx["nss"] = nssA[:, u : u + 1]
                else:
                    k = u - NCORES
                    ctx["off"] = 0
                    ctx["bt"] = bt89[k]
                    ctx["negsb"] = negsb89[k]
                    ctx["scal"] = lambda d, _k=k: m_row[
                        :, (NCORES + _k) * KD + d : (NCORES + _k) * KD + d + 1]
                    ctx["nscal"] = lambda d, _k=k: nscal89[
                        :, _k * KD + d : _k * KD + d + 1]
                    ctx["accum"] = cat[:, HID + u : HID + u + 1]
                    ctx["nss"] = nss89[:, k : k + 1]
                ctx["u"] = u
                ctx["w"] = N - ctx["off"]
                ctx["psl"] = ps_l1.tile([P, N], F32, tag="psl", name=f"psl_{u}")
                return ctx

            def relu_d(c, d, on_act=False):
                off = c["off"]
                at = apool.tile([P, N], BF16, tag="at", name=f"at{d}")
                if on_act:
                    # ACT Relu(x - s) with the negated per-partition bias
                    nc.scalar.activation(
                        at[:, off:N], c["bt"][:, d * N + off : (d + 1) * N],
                        AF.Relu, bias=c["nscal"](d), scale=1.0,
                    )
                else:
                    nc.vector.tensor_scalar(
                        out=at[:, off:N],
                        in0=c["bt"][:, d * N + off : (d + 1) * N],
                        scalar1=c["scal"](d),
                        scalar2=0.0,
                        op0=ALU.subtract,
                        op1=ALU.max,
                    )
                return at

            def stream(c, at, first):
                for ho, hsz in span_chunks(c["off"]):
                    nc.tensor.matmul(
                        c["psl"][:, ho : ho + hsz],
                        idb,
                        at[:, ho : ho + hsz],
                        start=first,
                        stop=False,
                    )

            def unit_tail(c):
                off, u = c["off"], c["u"]
                n_direct = KD - 2 * COMBINES
                for ci in range(COMBINES):
                    lo = n_direct + 2 * ci
                    a0, a1 = relu_d(c, lo), relu_d(c, lo + 1)
                    comb = cpool.tile([P, N], BF16, tag="comb")
                    nc.vector.tensor_add(
                        comb[:, off:N], a0[:, off:N], a1[:, off:N]
                    )
                    stream(c, comb, False)
                for ho, hsz in span_chunks(off):
                    nc.tensor.matmul(
                        c["psl"][:, ho : ho + hsz],
                        ones1,
                        c["negsb"][:, ho : ho + hsz],
                        start=False,
                        stop=True,
                    )
                escr = epool.tile([P, N], BF16, tag="escr")
                nc.scalar.activation(
                    escr[:, off:N], c["psl"][:, off:N], AF.Exp, bias=c["nss"],
                    scale=-2.0, accum_out=c["accum"],
                )
                escr_tiles[u] = escr

            def emit_units(ulist):
                # relu/stream emission interleaved across the group per d, so
                # the DVE queue never head-of-line blocks on a late broadcast
                # slice and each slice feeds every unit of the group
                ctxs = [unit_ctx(u) for u in ulist]
                n_direct = KD - 2 * COMBINES
                for d in range(n_direct):
                    for c in ctxs:
                        on_act = c["w"] >= ACT_W and d == n_direct - 1
                        stream(c, relu_d(c, d, on_act=on_act), d == 0)
                for c in ctxs:
                    unit_tail(c)

            pscol = ps_col.tile([1, N], F32, tag="pscol")

            def emit_colsum(u):
                # column sums of unit u's exp tile over its off-diag columns
                # [128(u+1), 1024); accumulated into the pscol row
                cs_off = (u + 1) * P
                escr = escr_tiles[u]
                for co, csz in span_chunks(cs_off):
                    lastu = 2 if co < 512 else NCORES - 2
                    nc.tensor.matmul(
                        pscol[0:1, co : co + csz],
                        onesc,
                        escr[:, co : co + csz],
                        start=(u == 0),
                        stop=(u == lastu),
                        skip_group_check=True,
                    )

            for u in range(NCORES):
                emit_units([u])
            for u in range(NCORES - 1):
                emit_colsum(u)
            # fold the transposed colsum pieces into divsend cols 1..7
            cs_sb = mtiles.tile([1, N], BF16, tag="cs_sb")
            nc.vector.tensor_copy(cs_sb[0:1, P:N], pscol[0:1, P:N])
            ps_ct = ps_prep.tile([128, 108], BF16, tag="ps_prep")
            for v in range(1, NCORES):
                nc.tensor.transpose(
                    ps_ct[:, 2 * (v - 1) : 2 * (v - 1) + 1],
                    cs_sb[0:1, v * P : (v + 1) * P],
                    idb[:1, :1],
                )
            nc.vector.tensor_tensor(
                out=divsend[:, 1:NCORES],
                in0=divsend[:, 1:NCORES],
                in1=bass.AP(tensor=ps_ct.tensor, offset=ps_ct.offset,
                            ap=[ps_ct.ap[0], [2, NCORES - 1]]),
                op=ALU.add,
            )

            # ---- exchange div columns (AllToAll): the send side fires as
            # soon as unit 7's exp lands, overlapping units 8/9 compute ----
            ps_ds = ps_small.tile([128, P], F32, tag="ps_small")
            nc.tensor.transpose(ps_ds[:NCORES, :], divsend, idf)
            dsend_sb = small.tile([NCORES, P], F32, tag="dsend")
            nc.vector.tensor_copy(dsend_sb, ps_ds[:NCORES, :])
            nc.sync.dma_start(out=a2a_send[b][:, :], in_=dsend_sb)
            if stage == "nocc":
                nc.gpsimd.dma_start(
                    out=a2a_recv[b][:, :], in_=a2a_send[b][:, :]
                )
            else:
                nc.gpsimd.collective_compute(
                    "AllToAll",
                    ALU.bypass,
                    replica_groups=[list(range(NCORES))],
                    ins=[a2a_send[b][:, :]],
                    outs=[a2a_recv[b][:, :]],
                )
            drecv_sb = small.tile([NCORES, P], F32, tag="drecv")
            nc.gpsimd.dma_start(out=drecv_sb, in_=a2a_recv[b][:, :])

            emit_units([NCORES])
            emit_units([NCORES + 1])

            ps_dr = ps_small.tile([128, NCORES], F32, tag="ps_small")
            nc.tensor.transpose(
                ps_dr[:, :NCORES], drecv_sb, idf[:NCORES, :NCORES]
            )
            nc.vector.tensor_copy(
                cat[:, HID : HID + NCORES], ps_dr[:, :NCORES]
            )

            # LayerNorm (center+scale, beta only). bn_stats in two groups:
            # the 256 h columns are ready mid-unit-loop, only the 10 div
            # columns land late
            stats = small.tile([P, 12], F32, tag="stats")
            nc.vector.bn_stats(out=stats[:, 0:6], in_=cat[:, :HID])
            nc.vector.bn_stats(out=stats[:, 6:12], in_=cat[:, HID:CAT])
            mv = small.tile([P, 2], F32, tag="mv")
            nc.vector.bn_aggr(out=mv, in_=stats)
            # rstd = 1/sqrt(var + eps) entirely on DVE via the bit-trick
            # seed + two Newton steps (keeps ACT on the Exp table; hardware
            # DVE has no sqrt/pow)
            ve = small.tile([P, 1], F32, tag="ve")
            nc.vector.tensor_scalar(
                out=ve, in0=mv[:, 1:2], scalar1=EPS, scalar2=None, op0=ALU.add
            )
            yi = small.tile([P, 1], F32, tag="rstd_y")
            nc.vector.tensor_scalar(
                out=yi.bitcast(mybir.dt.int32), in0=ve.bitcast(mybir.dt.int32),
                scalar1=1, scalar2=None, op0=ALU.arith_shift_right,
            )
            y0 = small.tile([P, 1], F32, tag="rstd_y0")
            nc.vector.tensor_scalar(
                out=y0.bitcast(mybir.dt.int32), in0=yi.bitcast(mybir.dt.int32),
                scalar1=-1, scalar2=0x5F3759DF, op0=ALU.mult, op1=ALU.add,
            )
            rstd = y0
            for _ in range(2):
                t2 = small.tile([P, 1], F32, tag="rstd_t2")
                nc.vector.tensor_tensor(out=t2, in0=rstd, in1=rstd, op=ALU.mult)
                nc.vector.tensor_tensor(out=t2, in0=t2, in1=ve, op=ALU.mult)
                nc.vector.tensor_scalar(
                    out=t2, in0=t2, scalar1=-0.5, scalar2=1.5,
                    op0=ALU.mult, op1=ALU.add,
                )
                ystep = small.tile([P, 1], F32, tag="rstd_ys")
                nc.vector.tensor_tensor(out=ystep, in0=rstd, in1=t2, op=ALU.mult)
                rstd = ystep
            catn = acts.tile([P, CAT], F32, name=f"catn{b}")
            nc.vector.tensor_scalar(
                out=catn,
                in0=cat,
                scalar1=mv[:, 0:1],
                scalar2=rstd,
                op0=ALU.subtract,
                op1=ALU.mult,
            )
            nc.vector.tensor_add(catn, catn, late_sb["beta"][b])
            # leaky relu in one fused op: max(0.3*x, x)
            hout = acts.tile([P, CAT], F32, name=f"hout{b}")
            nc.vector.scalar_tensor_tensor(
                out=hout, in0=catn, scalar=ALPHA, in1=catn,
                op0=ALU.mult, op1=ALU.max,
            )
            return hout

        # ---------- block 0 ----------
        prev0 = [(t, 128) for t in xT_sb]
        h1 = block(0, prev0, w0_sb, b0_sb, wd0_sb, bd0_sb,
                   after_ship=emit_late_consts)

        # transpose h1 -> feature-major bf16 chunks for block 1; all three
        # transposes land in one PSUM tile so a single copy drains them
        ps_t = ps_small.tile([128, 3 * P], F32, tag="ps_small")
        for ci, (co, csz) in enumerate(_chunks(CAT, 128)):
            nc.tensor.transpose(
                ps_t[:csz, ci * P : (ci + 1) * P], h1[:, co : co + csz], idf
            )
        h1T_all = acts.tile([128, 3 * P], BF16, name="h1T_all")
        nc.vector.tensor_copy(h1T_all, ps_t)
        h1T = [
            (h1T_all[:csz, ci * P : (ci + 1) * P], csz)
            for ci, (co, csz) in enumerate(_chunks(CAT, 128))
        ]

        # ---------- block 1 ----------
        h2 = block(1, h1T, late_sb["w1"], late_sb["b1"], late_sb["wd1"],
                   late_sb["bd1"])

        # ---------- critic head: y = h2 @ Wf + bf ----------
        hw = acts.tile([P, CAT], F32, name="hw")
        yacc = small.tile([P, 1], F32, tag="yacc")
        nc.vector.tensor_mul(hw, h2, late_sb["wf"])
        nc.vector.tensor_reduce(
            out=yacc, in_=hw, axis=mybir.AxisListType.X, op=ALU.add
        )
        ysb = small.tile([P, 1], F32, tag="ysb")
        nc.vector.tensor_scalar(
            out=ysb, in0=yacc, scalar1=late_sb["bf"], scalar2=None, op0=ALU.add
        )
        nc.sync.dma_start(out=y_out[:, :], in_=ysb)

        ps_l1.release()
        ps_col.release()
        ps_prep.release()
        ps_small.release()
        small.release()
        epool.release()
        cpool.release()
        apool.release()
        bpool.release()
        mtiles.release()
        acts.release()
        consts.release()
        dram.release()

    nc.compile()
    return nc


_NC_CACHE = {}


def _get_nc():
    stage = os.environ.get("KERNEL_STAGE", "full")
    if stage not in _NC_CACHE:
        _NC_CACHE[stage] = build_program(stage)
    return _NC_CACHE[stage]


def _make_in_maps(inputs):
    if BF16_NP is None:
        raise RuntimeError("ml_dtypes required for bf16 inputs")
    f = lambda a: np.ascontiguousarray(np.asarray(a, dtype=np.float32))
    bf = lambda a: np.ascontiguousarray(np.asarray(a, dtype=np.float32)).astype(
        BF16_NP
    )
    x = f(inputs["x"])
    shared = {
        "W0": bf(inputs["W0"]),
        "b0c": f(inputs["b0"]).reshape(HID, 1),
        "Wd0": bf(inputs["Wd0"]),
        "bd0c": f(inputs["bd0"]).reshape(MB, 1),
        "beta0b": np.ascontiguousarray(
            np.broadcast_to(f(inputs["beta0"]), (P, CAT))
        ),
        "W1": bf(inputs["W1"]),
        "b1c": f(inputs["b1"]).reshape(HID, 1),
        "Wd1": bf(inputs["Wd1"]),
        "bd1c": f(inputs["bd1"]).reshape(MB, 1),
        "beta1b": np.ascontiguousarray(
            np.broadcast_to(f(inputs["beta1"]), (P, CAT))
        ),
        "Wfb": np.ascontiguousarray(
            np.broadcast_to(f(inputs["Wf"]).reshape(1, CAT), (P, CAT))
        ),
        "bfc": np.full((P, 1), float(np.asarray(inputs["bf"]).reshape(-1)[0]),
                       dtype=np.float32),
    }
    in_maps = []
    for c in range(NCORES):
        m = dict(shared)
        m["xT"] = np.ascontiguousarray(x[c * P : (c + 1) * P, :].T).astype(
            BF16_NP
        )
        in_maps.append(m)
    return in_maps


def run(inputs, **kw):
    nc = _get_nc()
    in_maps = _make_in_maps(inputs)
    res = run_bass_kernel_spmd(nc, in_maps, list(range(NCORES)), **kw)
    y = np.concatenate([res.results[c]["y"] for c in range(NCORES)], axis=0)
    return y.astype(np.float32), res


def kernel(**inputs) -> np.ndarray:
    y, _ = run(inputs)
    return y
